# revision 1
# baseline (speedup 1.0000x reference)
"""Trainium2 Bass kernel for nn_AttentionModel (greedy pointer-attention decode).

Contract: kernel(**inputs) takes FULL inputs (B=1024), shards batch across 8
NeuronCores (128 items each, SPMD), runs the 199-step greedy decode on-device,
returns full (1024, 199, 200) float32 log_p.

Per-core dataflow (batch-on-partitions, b=128):
  precompute: emb2 = emb + pref -> DRAM;  kvl = emb2 @ W_node -> DRAM (gK|gV|lK)
              fixed2 = mean(emb2) @ W_fixed + first @ W_step[:256]
  per step  : stream kvl chunks from HBM; DVE does batched dot-products
              (multiply + strided reduce); ACT does exp/tanh/ln; PE does the
              shared-weight matmuls (cur @ W_step[256:], glimpse @ W_out) and
              transposes; argmax via DVE max/max_index; visited mask kept as a
              0/-1e9 addend; cur_emb gathered by indirect DMA with on-device
              computed row offsets.
"""
import numpy as np

import concourse.bass as bass
from concourse import bacc
import concourse.tile as tile
from concourse import mybir
from concourse.bass import IndirectOffsetOnAxis
from concourse.bass_utils import run_bass_kernel_spmd

dt = mybir.dt
F32 = dt.float32
AX = mybir.AxisListType
OP = mybir.AluOpType
ACTF = mybir.ActivationFunctionType

B, N, D, H = 1024, 200, 256, 8
d = D // H                      # 32
NCORES = 8
BS = B // NCORES                # 128 items per core
T = N - 1                       # 199 decode steps
START = 24
NEG = -1e9
CHUNK = 20                      # n-chunk for streaming kvl
NCH = N // CHUNK                # 10 chunks
ISD = 1.0 / np.sqrt(d).item()   # 1/sqrt(32)
ISD32 = float(np.float32(1.0 / np.sqrt(32.0)))
ISD256 = 0.0625                 # 1/sqrt(256), exact


def _build():
    nc = bacc.Bacc("TRN2", target_bir_lowering=False, debug=False)

    emb_in = nc.dram_tensor("embeddings", [BS, N, D], F32, kind="ExternalInput").ap()
    pref_in = nc.dram_tensor("pref_embed", [D], F32, kind="ExternalInput").ap()
    wnode_in = nc.dram_tensor("W_node", [D, 3 * D], F32, kind="ExternalInput").ap()
    wfix_in = nc.dram_tensor("W_fixed", [D, D], F32, kind="ExternalInput").ap()
    wstep_in = nc.dram_tensor("W_step", [2 * D, D], F32, kind="ExternalInput").ap()
    wout_in = nc.dram_tensor("W_out", [D, D], F32, kind="ExternalInput").ap()

    out = nc.dram_tensor("log_p", [BS, T * N], F32, kind="ExternalOutput").ap()

    emb2_d = nc.dram_tensor("emb2_d", [BS * N, D], F32).ap()
    kvl_d = nc.dram_tensor("kvl_d", [BS, N, 3 * D], F32).ap()

    with tile.TileContext(nc) as tc:
        with (
            tc.tile_pool(name="wpool", bufs=1) as wpool,      # persistent weights/state
            tc.tile_pool(name="stream", bufs=4) as stream,    # kvl chunks
            tc.tile_pool(name="prod", bufs=2) as prodp,       # TT products
            tc.tile_pool(name="work", bufs=2) as work,        # small transient tiles
            tc.tile_pool(name="psum", bufs=2, space="PSUM") as psp,
            tc.tile_pool(name="psum1", bufs=2, space="PSUM") as psp1,
        ):
            # ---------------- persistent tiles ----------------
            wn_sb = wpool.tile([128, 2, 3 * D], F32)    # W_node as [c-tile, 2, 768]
            nc.sync.dma_start(wn_sb[:, 0, :], wnode_in[0:128, :])
            nc.sync.dma_start(wn_sb[:, 1, :], wnode_in[128:256, :])
            w2_sb = wpool.tile([128, 2, D], F32)        # W_step[256:512] c-tiles
            nc.sync.dma_start(w2_sb[:, 0, :], wstep_in[256:384, :])
            nc.sync.dma_start(w2_sb[:, 1, :], wstep_in[384:512, :])
            wo_sb = wpool.tile([128, 2, D], F32)        # W_out c-tiles
            nc.sync.dma_start(wo_sb[:, 0, :], wout_in[0:128, :])
            nc.sync.dma_start(wo_sb[:, 1, :], wout_in[128:256, :])
            wf_sb = wpool.tile([128, 2, D], F32)        # W_fixed c-tiles
            nc.sync.dma_start(wf_sb[:, 0, :], wfix_in[0:128, :])
            nc.sync.dma_start(wf_sb[:, 1, :], wfix_in[128:256, :])
            ws1_sb = wpool.tile([128, 2, D], F32)       # W_step[0:256] c-tiles
            nc.sync.dma_start(ws1_sb[:, 0, :], wstep_in[0:128, :])
            nc.sync.dma_start(ws1_sb[:, 1, :], wstep_in[128:256, :])

            pref_sb = wpool.tile([128, D], F32)
            nc.sync.dma_start(
                pref_sb[:],
                pref_in.rearrange("(o f) -> o f", o=1).broadcast_to([128, D]),
            )

            ident = wpool.tile([128, 128], F32)         # identity for PE transpose
            io_c = wpool.tile([128, 128], dt.int32)
            nc.gpsimd.iota(io_c[:], pattern=[[1, 128]], channel_multiplier=0)
            io_r = wpool.tile([128, 1], dt.int32)
            nc.gpsimd.iota(io_r[:], pattern=[[0, 1]], channel_multiplier=1)
            id_i = wpool.tile([128, 128], dt.int32)
            nc.vector.tensor_tensor(id_i[:], io_c[:], io_r[:].broadcast_to([128, 128]), op=OP.is_equal)
            nc.vector.tensor_copy(ident[:], id_i[:])

            iota_n = wpool.tile([128, N], dt.int32)     # 0..199 per partition
            nc.gpsimd.iota(iota_n[:], pattern=[[1, N]], channel_multiplier=0)
            iota_row = wpool.tile([128, 1], dt.int32)   # p*N
            nc.gpsimd.iota(iota_row[:], pattern=[[0, 1]], channel_multiplier=N)

            amask = wpool.tile([128, N], F32)           # visited addend 0/-1e9
            nc.vector.memset(amask[:], 0.0)
            nc.vector.memset(amask[:, START:START + 1], NEG)

            fixed2 = wpool.tile([128, D], F32)
            first_sb = wpool.tile([128, D], F32)
            q_sb = wpool.tile([128, D], F32)
            cur_sb = wpool.tile([128, D], F32)

            # ---------------- precompute: emb2 + kvl ----------------
            emb_rows = emb_in.rearrange("b n c -> (b n) c")   # [25600, 256]
            ROWT = BS * N // 128                              # 200 row-tiles

            def pre_body(rt):
                erow = work.tile([128, D], F32, tag="erow")
                nc.sync.dma_start(erow[:], emb_rows[bass.ds(rt * 128, 128), :])
                e2 = work.tile([128, D], F32, tag="e2")
                nc.vector.tensor_tensor(e2[:], erow[:], pref_sb[:], op=OP.add)
                nc.sync.dma_start(emb2_d[bass.ds(rt * 128, 128), :], e2[:])
                # transpose e2 -> e2T (2 c-tiles)
                e2T = work.tile([128, 2, 128], F32, tag="e2T")
                for ci in range(2):
                    tp = psp1.tile([128, 128], F32, tag="tp")
                    nc.tensor.transpose(tp[:], e2[:, ci * 128:(ci + 1) * 128], ident[:])
                    nc.vector.tensor_copy(e2T[:, ci, :], tp[:])
                # kvl row-tile = e2 @ W_node  (f split 2x384)
                kv = work.tile([128, 3 * D], F32, tag="kv")
                for fh in range(2):
                    pm = psp.tile([128, 384], F32, tag="ps")
                    nc.tensor.matmul(pm[:], e2T[:, 0, :], wn_sb[:, 0, fh * 384:(fh + 1) * 384], start=True, stop=False)
                    nc.tensor.matmul(pm[:], e2T[:, 1, :], wn_sb[:, 1, fh * 384:(fh + 1) * 384], start=False, stop=True)
                    nc.vector.tensor_copy(kv[:, fh * 384:(fh + 1) * 384], pm[:])
                nc.sync.dma_start(kvl_d.rearrange("b n c -> (b n) c")[bass.ds(rt * 128, 128), :], kv[:])

            tc.For_i_unrolled(0, ROWT, 1, pre_body, max_unroll=4)

            # ---------------- fixed2 ----------------
            macc = wpool.tile([128, D], F32)
            emb2_bnc = emb2_d.rearrange("(b n) c -> b n c", b=BS)
            for c in range(NCH):
                ech = stream.tile([128, CHUNK, D], F32, tag="stream")
                nc.sync.dma_start(ech[:], emb2_bnc[:, c * CHUNK:(c + 1) * CHUNK, :])
                part = work.tile([128, D], F32, tag="mpart")
                nc.vector.tensor_reduce(part[:], ech[:].transpose([0, 2, 1]), axis=AX.X, op=OP.add)
                if c == 0:
                    nc.vector.tensor_copy(macc[:], part[:])
                else:
                    nc.vector.tensor_tensor(macc[:], macc[:], part[:], op=OP.add)
            nc.vector.tensor_scalar(macc[:], macc[:], 1.0 / N, None, op0=OP.mult)
            nc.sync.dma_start(first_sb[:], emb2_bnc[:, START, :])

            fT = work.tile([128, 2, 128], F32, tag="fT")
            mT = work.tile([128, 2, 128], F32, tag="mT")
            for ci in range(2):
                tp = psp1.tile([128, 128], F32, tag="tp")
                nc.tensor.transpose(tp[:], macc[:, ci * 128:(ci + 1) * 128], ident[:])
                nc.vector.tensor_copy(mT[:, ci, :], tp[:])
                tp2 = psp1.tile([128, 128], F32, tag="tp")
                nc.tensor.transpose(tp2[:], first_sb[:, ci * 128:(ci + 1) * 128], ident[:])
                nc.vector.tensor_copy(fT[:, ci, :], tp2[:])
            pf = psp.tile([128, D], F32, tag="ps")
            nc.tensor.matmul(pf[:], mT[:, 0, :], wf_sb[:, 0, :], start=True, stop=False)
            nc.tensor.matmul(pf[:], mT[:, 1, :], wf_sb[:, 1, :], start=False, stop=False)
            nc.tensor.matmul(pf[:], fT[:, 0, :], ws1_sb[:, 0, :], start=False, stop=False)
            nc.tensor.matmul(pf[:], fT[:, 1, :], ws1_sb[:, 1, :], start=False, stop=True)
            nc.vector.tensor_copy(fixed2[:], pf[:])

            # q(t=0): cur = first_emb
            nc.vector.tensor_copy(cur_sb[:], first_sb[:])

            def q_from_cur():
                cT = work.tile([128, 2, 128], F32, tag="cT")
                for ci in range(2):
                    tp = psp1.tile([128, 128], F32, tag="tp")
                    nc.tensor.transpose(tp[:], cur_sb[:, ci * 128:(ci + 1) * 128], ident[:])
                    nc.scalar.copy(cT[:, ci, :], tp[:])
                pq = psp.tile([128, D], F32, tag="ps")
                nc.tensor.matmul(pq[:], cT[:, 0, :], w2_sb[:, 0, :], start=True, stop=False)
                nc.tensor.matmul(pq[:], cT[:, 1, :], w2_sb[:, 1, :], start=False, stop=True)
                nc.scalar.activation(q_sb[:], pq[:], ACTF.Copy)
                nc.vector.tensor_tensor(q_sb[:], q_sb[:], fixed2[:], op=OP.add)

            q_from_cur()

            # ---------------- decode steps ----------------
            compat = wpool.tile([128, H, N], F32)
            attn = wpool.tile([128, H, N], F32)
            logits = wpool.tile([128, N], F32)
            gl_part = wpool.tile([128, NCH, D], F32)
            glimpse = wpool.tile([128, D], F32)

            def step_body(s):
                qb = q_sb[:].rearrange("p (o f) -> p o f", o=1).broadcast_to([128, CHUNK, D])
                # --- compat: per-head dots with gK ---
                for c in range(NCH):
                    kc = stream.tile([128, CHUNK, D], F32, tag="stream")
                    nc.sync.dma_start(kc[:], kvl_d[:, c * CHUNK:(c + 1) * CHUNK, 0:D])
                    pr = prodp.tile([128, CHUNK, D], F32, tag="prod")
                    nc.gpsimd.tensor_tensor(pr[:], kc[:], qb, op=OP.mult)
                    nc.vector.tensor_reduce(
                        compat[:, :, c * CHUNK:(c + 1) * CHUNK].transpose([0, 2, 1]),
                        pr[:].rearrange("p n (h e) -> p n h e", h=H),
                        axis=AX.X, op=OP.add)
                # scale + mask + softmax over n (per head)
                ab = amask[:].rearrange("p (o n) -> p o n", o=1).broadcast_to([128, H, N])
                nc.vector.tensor_scalar(compat[:], compat[:], ISD32, None, op0=OP.mult)
                nc.vector.tensor_tensor(compat[:], compat[:], ab, op=OP.add)
                mh = work.tile([128, H], F32, tag="mh")
                nc.vector.tensor_reduce(mh[:], compat[:], axis=AX.X, op=OP.max)
                nc.vector.tensor_tensor(
                    compat[:], compat[:],
                    mh[:].rearrange("p (h o) -> p h o", o=1).broadcast_to([128, H, N]),
                    op=OP.subtract)
                nc.scalar.activation(attn[:], compat[:], ACTF.Exp)
                sh = work.tile([128, H], F32, tag="sh")
                nc.vector.tensor_reduce(sh[:], attn[:], axis=AX.X, op=OP.add)
                rh = work.tile([128, H], F32, tag="rh")
                nc.vector.reciprocal(rh[:], sh[:])
                nc.vector.tensor_tensor(
                    attn[:], attn[:],
                    rh[:].rearrange("p (h o) -> p h o", o=1).broadcast_to([128, H, N]),
                    op=OP.mult)
                # --- glimpse: attn-weighted gV ---
                for c in range(NCH):
                    vc = stream.tile([128, CHUNK, D], F32, tag="stream")
                    nc.sync.dma_start(vc[:], kvl_d[:, c * CHUNK:(c + 1) * CHUNK, D:2 * D])
                    av = attn[:, :, c * CHUNK:(c + 1) * CHUNK].transpose([0, 2, 1]) \
                        .rearrange("p n (h o) -> p n h o", o=1).broadcast_to([128, CHUNK, H, d])
                    pr = prodp.tile([128, CHUNK, D], F32, tag="prod")
                    nc.gpsimd.tensor_tensor(pr[:].rearrange("p n (h e) -> p n h e", h=H), vc[:].rearrange("p n (h e) -> p n h e", h=H), av, op=OP.mult)
                    nc.vector.tensor_reduce(gl_part[:, c, :], pr[:].transpose([0, 2, 1]), axis=AX.X, op=OP.add)
                nc.vector.tensor_reduce(glimpse[:], gl_part[:].transpose([0, 2, 1]), axis=AX.X, op=OP.add)
                # g = glimpse @ W_out
                gT = work.tile([128, 2, 128], F32, tag="gT")
                for ci in range(2):
                    tp = psp1.tile([128, 128], F32, tag="tp")
                    nc.tensor.transpose(tp[:], glimpse[:, ci * 128:(ci + 1) * 128], ident[:])
                    nc.scalar.copy(gT[:, ci, :], tp[:])
                pg = psp.tile([128, D], F32, tag="ps")
                nc.tensor.matmul(pg[:], gT[:, 0, :], wo_sb[:, 0, :], start=True, stop=False)
                nc.tensor.matmul(pg[:], gT[:, 1, :], wo_sb[:, 1, :], start=False, stop=True)
                g_sb = work.tile([128, D], F32, tag="g_sb")
                nc.scalar.copy(g_sb[:], pg[:])
                gb = g_sb[:].rearrange("p (o f) -> p o f", o=1).broadcast_to([128, CHUNK, D])
                # --- logits: g . lK ---
                for c in range(NCH):
                    lc = stream.tile([128, CHUNK, D], F32, tag="stream")
                    nc.sync.dma_start(lc[:], kvl_d[:, c * CHUNK:(c + 1) * CHUNK, 2 * D:3 * D])
                    pr = prodp.tile([128, CHUNK, D], F32, tag="prod")
                    nc.gpsimd.tensor_tensor(pr[:], lc[:], gb, op=OP.mult)
                    nc.vector.tensor_reduce(logits[:, c * CHUNK:(c + 1) * CHUNK], pr[:], axis=AX.X, op=OP.add)
                # tanh clip, mask, log_softmax
                lgt = work.tile([128, N], F32, tag="lgt")
                nc.scalar.activation(lgt[:], logits[:], ACTF.Tanh, scale=ISD256)
                nc.vector.tensor_scalar(logits[:], lgt[:], 10.0, None, op0=OP.mult)
                nc.vector.tensor_tensor(logits[:], logits[:], amask[:], op=OP.add)
                m1 = work.tile([128, 1], F32, tag="m1")
                nc.vector.tensor_reduce(m1[:], logits[:], axis=AX.X, op=OP.max)
                shl = work.tile([128, N], F32, tag="shl")
                nc.vector.tensor_tensor(shl[:], logits[:], m1[:].broadcast_to([128, N]), op=OP.subtract)
                pexp = work.tile([128, N], F32, tag="pexp")
                s1 = work.tile([128, 1], F32, tag="s1")
                nc.scalar.activation(pexp[:], shl[:], ACTF.Exp, accum_out=s1[:])
                ls = work.tile([128, 1], F32, tag="ls")
                nc.scalar.activation(ls[:], s1[:], ACTF.Ln)
                lp = work.tile([128, N], F32, tag="lp")
                nc.vector.tensor_tensor(lp[:], shl[:], ls[:].broadcast_to([128, N]), op=OP.subtract)
                nc.sync.dma_start(out[:, bass.ds(s * N, N)], lp[:])
                # --- argmax + state update ---
                mx8 = work.tile([128, 8], F32, tag="mx8")
                nc.vector.max(mx8[:], logits[:])
                ix8 = work.tile([128, 8], dt.uint32, tag="ix8")
                nc.vector.max_index(ix8[:], mx8[:], logits[:])
                sel = work.tile([128, 1], dt.int32, tag="sel")
                nc.vector.tensor_copy(sel[:], ix8[:, 0:1])
                ohi = work.tile([128, N], dt.int32, tag="ohi")
                nc.vector.tensor_tensor(ohi[:], iota_n[:], sel[:].broadcast_to([128, N]), op=OP.is_equal)
                ohf = work.tile([128, N], F32, tag="ohf")
                nc.vector.tensor_copy(ohf[:], ohi[:])
                nc.vector.tensor_scalar(ohf[:], ohf[:], NEG, None, op0=OP.mult)
                nc.vector.tensor_tensor(amask[:], amask[:], ohf[:], op=OP.add)
                # gather next cur + q
                offs = work.tile([128, 1], dt.int32, tag="offs")
                nc.vector.tensor_tensor(offs[:], iota_row[:], sel[:], op=OP.add)
                nc.gpsimd.indirect_dma_start(
                    out=cur_sb[:], out_offset=None,
                    in_=emb2_d, in_offset=IndirectOffsetOnAxis(ap=offs[:], axis=0))
                q_from_cur()

            tc.For_i_unrolled(0, T, 1, step_body, max_unroll=4)

    nc.compile()
    return nc


_CACHE = {}


def kernel(**inputs) -> np.ndarray:
    if "nc" not in _CACHE:
        _CACHE["nc"] = _build()
    nc = _CACHE["nc"]

    emb = np.ascontiguousarray(np.asarray(inputs["embeddings"], np.float32))
    shared = {
        "pref_embed": np.asarray(inputs["pref_embed"], np.float32),
        "W_node": np.asarray(inputs["W_node"], np.float32),
        "W_fixed": np.asarray(inputs["W_fixed"], np.float32),
        "W_step": np.asarray(inputs["W_step"], np.float32),
        "W_out": np.asarray(inputs["W_out"], np.float32),
    }
    in_maps = []
    for i in range(NCORES):
        m = {"embeddings": emb[i * BS:(i + 1) * BS]}
        m.update(shared)
        in_maps.append(m)

    res = run_bass_kernel_spmd(nc, in_maps, list(range(NCORES)))
    outs = [res.results[i]["log_p"].reshape(BS, T, N) for i in range(NCORES)]
    return np.concatenate(outs, axis=0)


if __name__ == "__main__":
    z = np.load("inputs.npz")
    inp = {k: z[k] for k in z.files}
    o = kernel(**inp)
    print("kernel output", o.shape, o.dtype)
    np.save("kernel_out.npy", o)



# revision 13
# speedup vs baseline: 1.0041x; 1.0041x over previous
"""Trainium2 Bass kernel for nn_AttentionModel (greedy pointer-attention decode).

Contract: kernel(**inputs) takes FULL inputs (B=1024), shards batch across 8
NeuronCores (128 items each, SPMD), runs the 199-step greedy decode on-device,
returns full (1024, 199, 200) float32 log_p.

v2 design (all per-step matmuls algebraically folded away; DVE-centric):
  precompute (row-tile loop over (b,n) rows):
    e2 = emb + pref
    [gK | gV | lK] = e2 @ W_node  (PE fp32)
    K  = gK * isqrt(32)  -> bf16 rows, (d,h)-major cols -> DRAM -> resident SBUF
    V  = gV              -> bf16 rows, (d,h)-major cols -> DRAM (streamed)
    lK'= (lK @ W_out^T) * isqrt(256) -> bf16 (d,h)-major -> DRAM (streamed)
    S  = e2 @ W_step[256:] ((d,h)-major cols) -> DRAM f32 (gathered per step)
    fixed2 = mean(e2) @ W_fixed + e2[:,24] @ W_step[:256]  ((d,h)-major)
  per step (no matmuls at all; bf16 products + pairwise halving trees on DVE):
    q = fixed2 + S[prev]
    compat[n,h] = sum_e K[n,e,h]*q[e,h]
    attn = exp(compat+amask)/sum        (no max-sub; bounded by construction)
    glimpse[e,h] = sum_n V[n,e,h]*attn[n,h]   (streamed V chunks)
    logits[n] = sum_c lK'[n,c]*g[c]           (streamed lK' chunks)
    lm = 10*tanh(logits) + amask ; store row (host adds -logsumexp later)
    sel = argmax (DVE max/max_index); amask update; next q gather.
"""
import numpy as np

import concourse.bass as bass
from concourse import bacc
import concourse.tile as tile
from concourse import mybir
from concourse.bass import IndirectOffsetOnAxis
from concourse.bass_utils import run_bass_kernel_spmd

dt = mybir.dt
F32 = dt.float32
BF16 = dt.bfloat16
AX = mybir.AxisListType
OP = mybir.AluOpType
ACTF = mybir.ActivationFunctionType

B, N, D, H = 1024, 200, 256, 8
dd = D // H                     # 32
NCORES = 8
BS = B // NCORES                # 128 items per core
T = N - 1                       # 199 decode steps
START = 24
NEG = -1e9
NC = 20                         # n-chunk size
NCH = N // NC                   # 10 chunks
MC = 10                         # mean-pass chunk
ISD = float(np.float32(1.0 / np.sqrt(32.0)))
ISD256 = 0.0625
ROWT = BS * N // 128            # 200 row-tiles in precompute
DBG = False


def _build():
    nc = bacc.Bacc("TRN2", target_bir_lowering=False, debug=False)

    emb_in = nc.dram_tensor("embeddings", [BS, N, D], F32, kind="ExternalInput").ap()
    pref_in = nc.dram_tensor("pref_embed", [D], F32, kind="ExternalInput").ap()
    wnode_in = nc.dram_tensor("W_node", [D, 3 * D], F32, kind="ExternalInput").ap()
    wfix_in = nc.dram_tensor("W_fixed", [D, D], F32, kind="ExternalInput").ap()
    wstep_in = nc.dram_tensor("W_step", [2 * D, D], F32, kind="ExternalInput").ap()
    wout_in = nc.dram_tensor("W_out", [D, D], F32, kind="ExternalInput").ap()

    out = nc.dram_tensor("log_p", [BS, T * N], F32, kind="ExternalOutput").ap()

    if DBG:
        dbg_fixed2 = nc.dram_tensor("dbg_fixed2", [BS, D], F32, kind="ExternalOutput").ap()
        dbg_q = nc.dram_tensor("dbg_q", [BS, D], F32, kind="ExternalOutput").ap()
        dbg_compat = nc.dram_tensor("dbg_compat", [BS, N * H], F32, kind="ExternalOutput").ap()
        dbg_attn = nc.dram_tensor("dbg_attn", [BS, N * H], F32, kind="ExternalOutput").ap()
        dbg_g = nc.dram_tensor("dbg_g", [BS, D], F32, kind="ExternalOutput").ap()
        dbg_logits = nc.dram_tensor("dbg_logits", [BS, N], F32, kind="ExternalOutput").ap()
        dbg_sel = nc.dram_tensor("dbg_sel", [BS, 1], dt.int32, kind="ExternalOutput").ap()
        dbg_krow = nc.dram_tensor("dbg_krow", [BS, D], F32, kind="ExternalOutput").ap()
        dbg_lrow = nc.dram_tensor("dbg_lrow", [BS, D], F32, kind="ExternalOutput").ap()
        dbg_srow = nc.dram_tensor("dbg_srow", [BS, D], F32, kind="ExternalOutput").ap()
    e2_d = nc.dram_tensor("e2_d", [BS * N, D], F32).ap()
    krow_d = nc.dram_tensor("krow_d", [BS * N, D], F32).ap()
    vrow_d = nc.dram_tensor("vrow_d", [BS * N, D], F32).ap()
    lrow_d = nc.dram_tensor("lrow_d", [BS * N, D], F32).ap()
    s_d = nc.dram_tensor("s_d", [BS * N, D], F32).ap()

    with tile.TileContext(nc) as tc:
        with (
            tc.tile_pool(name="wpool", bufs=1) as wpool,
            tc.tile_pool(name="stream", bufs=4) as stream,
            tc.tile_pool(name="prod", bufs=2) as prodp,
            tc.tile_pool(name="work", bufs=2) as work,
            tc.tile_pool(name="small", bufs=1) as small,
        ):
          with (
            tc.tile_pool(name="wpre", bufs=1) as wpre,
            tc.tile_pool(name="pwork", bufs=1) as pwork,
            tc.tile_pool(name="psA", bufs=2, space="PSUM") as psA,
            tc.tile_pool(name="psB", bufs=2, space="PSUM") as psB,
            tc.tile_pool(name="psT", bufs=2, space="PSUM") as psT,
          ):
            # ---------------- precompute-only weights ----------------
            wn_sb = wpre.tile([128, 2, 3 * D], F32)
            nc.sync.dma_start(wn_sb[:, 0, :], wnode_in[0:128, :])
            nc.sync.dma_start(wn_sb[:, 1, :], wnode_in[128:256, :])
            ws2_sb = wpre.tile([128, 2, D], F32)       # W_step[256:512]
            nc.sync.dma_start(ws2_sb[:, 0, :], wstep_in[256:384, :])
            nc.sync.dma_start(ws2_sb[:, 1, :], wstep_in[384:512, :])
            ws1_sb = wpre.tile([128, 2, D], F32)       # W_step[0:256]
            nc.sync.dma_start(ws1_sb[:, 0, :], wstep_in[0:128, :])
            nc.sync.dma_start(ws1_sb[:, 1, :], wstep_in[128:256, :])
            wf_sb = wpre.tile([128, 2, D], F32)
            nc.sync.dma_start(wf_sb[:, 0, :], wfix_in[0:128, :])
            nc.sync.dma_start(wf_sb[:, 1, :], wfix_in[128:256, :])
            wo_sb = wpre.tile([128, 2, D], F32)        # W_out row-tiles
            nc.sync.dma_start(wo_sb[:, 0, :], wout_in[0:128, :])
            nc.sync.dma_start(wo_sb[:, 1, :], wout_in[128:256, :])

            pref_sb = wpre.tile([128, D], F32)
            nc.sync.dma_start(
                pref_sb[:],
                pref_in.rearrange("(o f) -> o f", o=1).broadcast_to([128, D]),
            )

            ident = wpre.tile([128, 128], F32)
            io_c = wpre.tile([128, 128], dt.int32)
            nc.gpsimd.iota(io_c[:], pattern=[[1, 128]], channel_multiplier=0)
            io_r = wpre.tile([128, 1], dt.int32)
            nc.gpsimd.iota(io_r[:], pattern=[[0, 1]], channel_multiplier=1)
            id_i = wpre.tile([128, 128], dt.int32)
            nc.vector.tensor_tensor(id_i[:], io_c[:], io_r[:].broadcast_to([128, 128]), op=OP.is_equal)
            nc.vector.tensor_copy(ident[:], id_i[:])

            iota_n = wpool.tile([128, N], dt.int32)
            nc.gpsimd.iota(iota_n[:], pattern=[[1, N]], channel_multiplier=0)
            iota_row = wpool.tile([128, 1], dt.int32)   # p*N
            nc.gpsimd.iota(iota_row[:], pattern=[[0, 1]], channel_multiplier=N)

            amask = wpool.tile([128, N], F32)
            nc.vector.memset(amask[:], 0.0)
            nc.vector.memset(amask[:, START:START + 1], NEG)

            sel = wpool.tile([128, 1], dt.int32)
            selF = wpool.tile([128, 1], F32)
            nc.vector.memset(selF[:], float(START))
            nc.vector.tensor_copy(sel[:], selF[:])

            fixed2 = wpool.tile([128, D], F32)

            # W_out^T in SBUF: woT2[:, jt, i] = W_out[i, j]
            woT2 = wpre.tile([128, 2, D], F32)
            for jt in range(2):
                for it in range(2):
                    tp = psT.tile([128, 128], F32, tag="tp")
                    nc.tensor.transpose(tp[:], wo_sb[:, it, jt * 128:(jt + 1) * 128], ident[:])
                    nc.scalar.copy(woT2[:, jt, it * 128:(it + 1) * 128], tp[:])


            # ---------------- precompute row-tile loop ----------------
            emb_rows = emb_in.rearrange("b n c -> (b n) c")

            def dhv(t2):
                # natural (h,e)-major [p, 256] -> [p, e, h] view (reorder cols)
                return t2.rearrange("p (h e) -> p e h", h=H)

            def ehs(t2):
                # contiguous (e,h)-major [p, 256] -> [p, e, h] view (plain split)
                return t2.rearrange("p (e h) -> p e h", h=H)

            def pre_body(rt):
                r0 = rt * 128
                e2 = pwork.tile([128, D], F32, tag="e2")
                nc.sync.dma_start(e2[:], emb_rows[bass.ds(r0, 128), :])
                nc.vector.tensor_tensor(e2[:], e2[:], pref_sb[:], op=OP.add)
                nc.sync.dma_start(e2_d[bass.ds(r0, 128), :], e2[:])
                e2T = pwork.tile([128, 2, 128], F32, tag="e2T")
                for ci in range(2):
                    tp = psT.tile([128, 128], F32, tag="tp")
                    nc.tensor.transpose(tp[:], e2[:, ci * 128:(ci + 1) * 128], ident[:])
                    nc.scalar.copy(e2T[:, ci, :], tp[:])
                # kvl = e2 @ W_node : psum [512] + [256]
                pa = psA.tile([128, 512], F32, tag="pa")
                nc.tensor.matmul(pa[:], e2T[:, 0, :], wn_sb[:, 0, 0:512], start=True, stop=False)
                nc.tensor.matmul(pa[:], e2T[:, 1, :], wn_sb[:, 1, 0:512], start=False, stop=True)
                pb = psB.tile([128, D], F32, tag="pbx")
                nc.tensor.matmul(pb[:], e2T[:, 0, :], wn_sb[:, 0, 512:768], start=True, stop=False)
                nc.tensor.matmul(pb[:], e2T[:, 1, :], wn_sb[:, 1, 512:768], start=False, stop=True)
                # K row (scaled, (d,h)-major) and V row
                krow = pwork.tile([128, D], F32, tag="krow")
                nc.scalar.activation(krow[:], pa[:, 0:256], ACTF.Copy, scale=ISD)
                nc.sync.dma_start(krow_d[bass.ds(r0, 128), :], krow[:])
                vrow = pwork.tile([128, D], F32, tag="krow")
                nc.vector.tensor_copy(vrow[:], pa[:, 256:512])
                nc.sync.dma_start(vrow_d[bass.ds(r0, 128), :], vrow[:])
                # lK' = (lK @ W_out^T) * ISD256, (d,h)-major via rhs view
                lrow = pwork.tile([128, D], F32, tag="lrow")
                nc.scalar.copy(lrow[:], pb[:])
                lrT = pwork.tile([128, 2, 128], F32, tag="lrT")
                for ci in range(2):
                    tp = psT.tile([128, 128], F32, tag="tp")
                    nc.tensor.transpose(tp[:], lrow[:, ci * 128:(ci + 1) * 128], ident[:])
                    nc.scalar.copy(lrT[:, ci, :], tp[:])
                pc = psB.tile([128, D], F32, tag="pbx")
                nc.tensor.matmul(pc[:], lrT[:, 0, :], woT2[:, 0, :], start=True, stop=False)
                nc.tensor.matmul(pc[:], lrT[:, 1, :], woT2[:, 1, :], start=False, stop=True)
                lprow = pwork.tile([128, D], F32, tag="krow")
                nc.scalar.activation(lprow[:], pc[:], ACTF.Copy, scale=ISD256)
                nc.sync.dma_start(lrow_d[bass.ds(r0, 128), :], lprow[:])
                # S row = e2 @ W_step[256:], (d,h)-major
                pdm = psB.tile([128, D], F32, tag="pbx")
                nc.tensor.matmul(pdm[:], e2T[:, 0, :], ws2_sb[:, 0, :], start=True, stop=False)
                nc.tensor.matmul(pdm[:], e2T[:, 1, :], ws2_sb[:, 1, :], start=False, stop=True)
                srow = pwork.tile([128, D], F32, tag="lrow")
                nc.vector.tensor_copy(srow[:], pdm[:])
                nc.sync.dma_start(s_d[bass.ds(r0, 128), :], srow[:])

            tc.For_i_unrolled(0, ROWT, 1, pre_body, max_unroll=2)

            # -------- fixed2 = mean(e2) @ Wf + e2[:,24] @ Ws1, (d,h)-major --------
            macc = wpool.tile([128, D], F32)
            e2_bnc = e2_d.rearrange("(b n) c -> b n c", b=BS)
            for c in range(N // MC):
                ech = stream.tile([128, MC, D], F32, tag="ch")
                nc.sync.dma_start(ech[:], e2_bnc[:, c * MC:(c + 1) * MC, :])
                part = pwork.tile([128, D], F32, tag="e2")
                nc.vector.tensor_reduce(part[:], ech[:].transpose([0, 2, 1]), axis=AX.X, op=OP.add)
                if c == 0:
                    nc.vector.tensor_copy(macc[:], part[:])
                else:
                    nc.vector.tensor_tensor(macc[:], macc[:], part[:], op=OP.add)
            nc.vector.tensor_scalar(macc[:], macc[:], 1.0 / N, None, op0=OP.mult)
            first_sb = wpool.tile([128, D], F32)
            nc.sync.dma_start(first_sb[:], e2_bnc[:, START, :])

            fT = pwork.tile([128, 2, 128], F32, tag="e2T")
            mT = pwork.tile([128, 2, 128], F32, tag="lrT")
            for ci in range(2):
                tp = psT.tile([128, 128], F32, tag="tp")
                nc.tensor.transpose(tp[:], macc[:, ci * 128:(ci + 1) * 128], ident[:])
                nc.scalar.copy(mT[:, ci, :], tp[:])
                tp2 = psT.tile([128, 128], F32, tag="tp")
                nc.tensor.transpose(tp2[:], first_sb[:, ci * 128:(ci + 1) * 128], ident[:])
                nc.scalar.copy(fT[:, ci, :], tp2[:])
            pf = psA.tile([128, 512], F32, tag="pa")
            nc.tensor.matmul(pf[:, 0:256], mT[:, 0, :], wf_sb[:, 0, :], start=True, stop=False)
            nc.tensor.matmul(pf[:, 0:256], mT[:, 1, :], wf_sb[:, 1, :], start=False, stop=False)
            nc.tensor.matmul(pf[:, 0:256], fT[:, 0, :], ws1_sb[:, 0, :], start=False, stop=False)
            nc.tensor.matmul(pf[:, 0:256], fT[:, 1, :], ws1_sb[:, 1, :], start=False, stop=True)
            nc.vector.tensor_copy(fixed2[:], pf[:, 0:256])

            krow_bn = krow_d.rearrange("(b n) c -> b n c", b=BS)

          # ---------------- decode steps (fp32; DVE/POOL split) ----------------
          if True:
            compat = wpool.tile([128, N, H], F32)
            attn = wpool.tile([128, N, H], F32)
            logits = wpool.tile([128, N], F32)
            gacc = wpool.tile([128, D], F32)
            qf = wpool.tile([128, D], F32)

            vrow_bn = vrow_d.rearrange("(b n) c -> b n c", b=BS)
            lrow_bn = lrow_d.rearrange("(b n) c -> b n c", b=BS)
            NDV = 5          # chunks 0..NDV-1 on DVE, rest on POOL

            def eng(c):
                return nc.vector if c < NDV else nc.gpsimd

            def step_body(t):
                # q = fixed2 + S[prev]
                offs = small.tile([128, 1], dt.int32, tag="offs")
                nc.vector.tensor_tensor(offs[:], iota_row[:], sel[:], op=OP.add)
                srow = small.tile([128, D], F32, tag="sgath")
                nc.gpsimd.indirect_dma_start(
                    out=srow[:], out_offset=None,
                    in_=s_d, in_offset=IndirectOffsetOnAxis(ap=offs[:], axis=0))
                nc.vector.tensor_tensor(qf[:], fixed2[:], srow[:], op=OP.add)
                qbb = qf[:].rearrange("p (n c) -> p n c", n=1).broadcast_to([128, NC, D])

                # ---- compat[n,h] = sum_e K[n,h,e]*q[h,e] ----
                for c in range(NCH):
                    n0 = c * NC
                    kch = stream.tile([128, NC, D], F32, tag="ch")
                    nc.sync.dma_start(kch[:], krow_bn[:, n0:n0 + NC, :])
                    pr = prodp.tile([128, NC, D], F32, tag="pr")
                    eng(c).tensor_tensor(pr[:], kch[:], qbb, op=OP.mult)
                    nc.vector.tensor_reduce(
                        compat[:, n0:n0 + NC, :],
                        pr[:].rearrange("p n (h e) -> p n h e", h=H),
                        axis=AX.X, op=OP.add)
                # softmax over n per h (max-sub for safety)
                nc.vector.tensor_tensor(
                    compat[:], compat[:],
                    amask[:].rearrange("p (n o) -> p n o", o=1).broadcast_to([128, N, H]),
                    op=OP.add)
                cmax = small.tile([128, H], F32, tag="cmax")
                nc.vector.tensor_reduce(cmax[:], compat[:].transpose([0, 2, 1]), axis=AX.X, op=OP.max)
                nc.vector.tensor_tensor(
                    compat[:], compat[:],
                    cmax[:].rearrange("p (o h) -> p o h", o=1).broadcast_to([128, N, H]),
                    op=OP.subtract)
                nc.scalar.activation(attn[:], compat[:], ACTF.Exp)
                ssum = small.tile([128, H], F32, tag="ssum")
                nc.vector.tensor_reduce(ssum[:], attn[:].transpose([0, 2, 1]), axis=AX.X, op=OP.add)
                rh = small.tile([128, H], F32, tag="rh")
                nc.vector.reciprocal(rh[:], ssum[:])
                nc.vector.tensor_tensor(
                    attn[:], attn[:],
                    rh[:].rearrange("p (o h) -> p o h", o=1).broadcast_to([128, N, H]),
                    op=OP.mult)

                # ---- glimpse[h,e] = sum_n V[n,h,e]*attn[n,h] ----
                for c in range(NCH):
                    n0 = c * NC
                    vch = stream.tile([128, NC, D], F32, tag="ch")
                    nc.sync.dma_start(vch[:], vrow_bn[:, n0:n0 + NC, :])
                    pr2 = prodp.tile([128, NC, D], F32, tag="pr")
                    eng(c).tensor_tensor(
                        pr2[:].rearrange("p n (h e) -> p n h e", h=H),
                        vch[:].rearrange("p n (h e) -> p n h e", h=H),
                        attn[:, n0:n0 + NC, :].rearrange("p n (h o) -> p n h o", o=1)
                            .broadcast_to([128, NC, H, dd]),
                        op=OP.mult)
                    gpart = small.tile([128, D], F32, tag="gpart")
                    nc.vector.tensor_reduce(gpart[:], pr2[:].transpose([0, 2, 1]), axis=AX.X, op=OP.add)
                    if c == 0:
                        nc.vector.tensor_copy(gacc[:], gpart[:])
                    else:
                        nc.vector.tensor_tensor(gacc[:], gacc[:], gpart[:], op=OP.add)
                gbb = gacc[:].rearrange("p (n c) -> p n c", n=1).broadcast_to([128, NC, D])

                # ---- logits[n] = sum_c lK'[n,c]*g[c] ----
                for c in range(NCH):
                    n0 = c * NC
                    lch = stream.tile([128, NC, D], F32, tag="ch")
                    nc.sync.dma_start(lch[:], lrow_bn[:, n0:n0 + NC, :])
                    pr3 = prodp.tile([128, NC, D], F32, tag="pr")
                    eng(c).tensor_tensor(pr3[:], lch[:], gbb, op=OP.mult)
                    nc.vector.tensor_reduce(
                        logits[:, n0:n0 + NC], pr3[:], axis=AX.X, op=OP.add)

                # ---- tanh clip, mask, store (host does -logsumexp) ----
                tnh = work.tile([128, N], F32, tag="tnh")
                nc.scalar.activation(tnh[:], logits[:], ACTF.Tanh)
                lm = work.tile([128, N], F32, tag="lm")
                nc.vector.tensor_scalar(lm[:], tnh[:], 10.0, None, op0=OP.mult)
                nc.vector.tensor_tensor(lm[:], lm[:], amask[:], op=OP.add)
                nc.sync.dma_start(out[:, bass.ds(t * N, N)], lm[:])

                # ---- argmax + state update ----
                mx8 = small.tile([128, 8], F32, tag="mx8")
                nc.vector.max(mx8[:], lm[:])
                ix8 = small.tile([128, 8], dt.uint32, tag="ix8")
                nc.vector.max_index(ix8[:], mx8[:], lm[:])
                nc.vector.tensor_copy(sel[:], ix8[:, 0:1])
                ohi = small.tile([128, N], dt.int32, tag="ohi")
                nc.vector.tensor_tensor(ohi[:], iota_n[:], sel[:].broadcast_to([128, N]), op=OP.is_equal)
                ohf = small.tile([128, N], F32, tag="ohf")
                nc.vector.tensor_copy(ohf[:], ohi[:])
                nc.vector.scalar_tensor_tensor(
                    amask[:], ohf[:], NEG, amask[:], op0=OP.mult, op1=OP.add)

            if DBG:
                step_body(0)
                nc.sync.dma_start(dbg_fixed2[:, :], fixed2[:])
                nc.sync.dma_start(dbg_q[:, :], qf[:])
                nc.sync.dma_start(dbg_compat[:, :], compat[:].rearrange("p n h -> p (n h)"))
                nc.vector.tensor_copy(compat[:], attn[:])
                nc.sync.dma_start(dbg_attn[:, :], compat[:].rearrange("p n h -> p (n h)"))
                nc.vector.tensor_copy(qf[:], gacc[:])
                nc.sync.dma_start(dbg_g[:, :], qf[:])
                nc.sync.dma_start(dbg_logits[:, :], logits[:])
                nc.sync.dma_start(dbg_sel[:, :], sel[:])
                kch0 = stream.tile([128, NC, D], F32, tag="ch")
                nc.sync.dma_start(kch0[:], krow_bn[:, 20:40, :])
                nc.vector.tensor_copy(qf[:], kch0[:, 10, :])
                nc.sync.dma_start(dbg_krow[:, :], qf[:])
                lch0 = stream.tile([128, NC, D], F32, tag="ch")
                nc.sync.dma_start(lch0[:], lrow_bn[:, 20:40, :])
                nc.vector.tensor_copy(qf[:], lch0[:, 10, :])
                nc.sync.dma_start(dbg_lrow[:, :], qf[:])
                sr0 = small.tile([128, D], F32, tag="sgath")
                nc.sync.dma_start(sr0[:], s_d.rearrange("(b n) c -> b n c", b=BS)[:, 24, :])
                nc.sync.dma_start(dbg_srow[:, :], sr0[:])
            else:
                tc.For_i_unrolled(0, T, 1, step_body, max_unroll=2)

    nc.compile()
    return nc


_CACHE = {}


def kernel(**inputs) -> np.ndarray:
    if "nc" not in _CACHE:
        _CACHE["nc"] = _build()
    nc = _CACHE["nc"]

    emb = np.ascontiguousarray(np.asarray(inputs["embeddings"], np.float32))
    shared = {
        "pref_embed": np.asarray(inputs["pref_embed"], np.float32),
        "W_node": np.asarray(inputs["W_node"], np.float32),
        "W_fixed": np.asarray(inputs["W_fixed"], np.float32),
        "W_step": np.asarray(inputs["W_step"], np.float32),
        "W_out": np.asarray(inputs["W_out"], np.float32),
    }
    in_maps = []
    for i in range(NCORES):
        m = {"embeddings": emb[i * BS:(i + 1) * BS]}
        m.update(shared)
        in_maps.append(m)

    res = run_bass_kernel_spmd(nc, in_maps, list(range(NCORES)))
    outs = [res.results[i]["log_p"].reshape(BS, T, N) for i in range(NCORES)]
    lm = np.concatenate(outs, axis=0)  # (B, T, N): 10*tanh + mask, pre-normalization
    # host-side log_softmax normalization (exact, float64)
    x = lm.astype(np.float64)
    xf = np.where(x > -1e8, x, -np.inf)
    mx = xf.max(axis=2, keepdims=True)
    lse = mx + np.log(np.exp(xf - mx).sum(axis=2, keepdims=True))
    return (x - lse).astype(np.float32)


if __name__ == "__main__":
    z = np.load("inputs.npz")
    inp = {k: z[k] for k in z.files}
    o = kernel(**inp)
    print("kernel output", o.shape, o.dtype)
    np.save("kernel_out.npy", o)


# revision 14
# speedup vs baseline: 1.0731x; 1.0687x over previous
"""Trainium2 Bass kernel for nn_AttentionModel (greedy pointer-attention decode).

Contract: kernel(**inputs) takes FULL inputs (B=1024), shards batch across 8
NeuronCores (128 items each, SPMD), runs the 199-step greedy decode on-device,
returns full (1024, 199, 200) float32 log_p.

v2 design (all per-step matmuls algebraically folded away; DVE-centric):
  precompute (row-tile loop over (b,n) rows):
    e2 = emb + pref
    [gK | gV | lK] = e2 @ W_node  (PE fp32)
    K  = gK * isqrt(32)  -> bf16 rows, (d,h)-major cols -> DRAM -> resident SBUF
    V  = gV              -> bf16 rows, (d,h)-major cols -> DRAM (streamed)
    lK'= (lK @ W_out^T) * isqrt(256) -> bf16 (d,h)-major -> DRAM (streamed)
    S  = e2 @ W_step[256:] ((d,h)-major cols) -> DRAM f32 (gathered per step)
    fixed2 = mean(e2) @ W_fixed + e2[:,24] @ W_step[:256]  ((d,h)-major)
  per step (no matmuls at all; bf16 products + pairwise halving trees on DVE):
    q = fixed2 + S[prev]
    compat[n,h] = sum_e K[n,e,h]*q[e,h]
    attn = exp(compat+amask)/sum        (no max-sub; bounded by construction)
    glimpse[e,h] = sum_n V[n,e,h]*attn[n,h]   (streamed V chunks)
    logits[n] = sum_c lK'[n,c]*g[c]           (streamed lK' chunks)
    lm = 10*tanh(logits) + amask ; store row (host adds -logsumexp later)
    sel = argmax (DVE max/max_index); amask update; next q gather.
"""
import numpy as np

import concourse.bass as bass
from concourse import bacc
import concourse.tile as tile
from concourse import mybir
from concourse.bass import IndirectOffsetOnAxis
from concourse.bass_utils import run_bass_kernel_spmd

dt = mybir.dt
F32 = dt.float32
BF16 = dt.bfloat16
AX = mybir.AxisListType
OP = mybir.AluOpType
ACTF = mybir.ActivationFunctionType

B, N, D, H = 1024, 200, 256, 8
dd = D // H                     # 32
NCORES = 8
BS = B // NCORES                # 128 items per core
T = N - 1                       # 199 decode steps
START = 24
NEG = -1e9
NC = 20                         # n-chunk size
NCH = N // NC                   # 10 chunks
MC = 10                         # mean-pass chunk
ISD = float(np.float32(1.0 / np.sqrt(32.0)))
ISD256 = 0.0625
ROWT = BS * N // 128            # 200 row-tiles in precompute
DBG = False


def _build():
    nc = bacc.Bacc("TRN2", target_bir_lowering=False, debug=False)

    emb_in = nc.dram_tensor("embeddings", [BS, N, D], F32, kind="ExternalInput").ap()
    pref_in = nc.dram_tensor("pref_embed", [D], F32, kind="ExternalInput").ap()
    wnode_in = nc.dram_tensor("W_node", [D, 3 * D], F32, kind="ExternalInput").ap()
    wfix_in = nc.dram_tensor("W_fixed", [D, D], F32, kind="ExternalInput").ap()
    wstep_in = nc.dram_tensor("W_step", [2 * D, D], F32, kind="ExternalInput").ap()
    wout_in = nc.dram_tensor("W_out", [D, D], F32, kind="ExternalInput").ap()

    out = nc.dram_tensor("log_p", [BS, T * N], F32, kind="ExternalOutput").ap()

    if DBG:
        dbg_fixed2 = nc.dram_tensor("dbg_fixed2", [BS, D], F32, kind="ExternalOutput").ap()
        dbg_q = nc.dram_tensor("dbg_q", [BS, D], F32, kind="ExternalOutput").ap()
        dbg_compat = nc.dram_tensor("dbg_compat", [BS, N * H], F32, kind="ExternalOutput").ap()
        dbg_attn = nc.dram_tensor("dbg_attn", [BS, N * H], F32, kind="ExternalOutput").ap()
        dbg_g = nc.dram_tensor("dbg_g", [BS, D], F32, kind="ExternalOutput").ap()
        dbg_logits = nc.dram_tensor("dbg_logits", [BS, N], F32, kind="ExternalOutput").ap()
        dbg_sel = nc.dram_tensor("dbg_sel", [BS, 1], dt.int32, kind="ExternalOutput").ap()
        dbg_krow = nc.dram_tensor("dbg_krow", [BS, D], F32, kind="ExternalOutput").ap()
        dbg_lrow = nc.dram_tensor("dbg_lrow", [BS, D], F32, kind="ExternalOutput").ap()
        dbg_srow = nc.dram_tensor("dbg_srow", [BS, D], F32, kind="ExternalOutput").ap()
    e2_d = nc.dram_tensor("e2_d", [BS * N, D], F32).ap()
    krow_d = nc.dram_tensor("krow_d", [BS * N, D], F32).ap()
    vrow_d = nc.dram_tensor("vrow_d", [BS * N, D], F32).ap()
    lrow_d = nc.dram_tensor("lrow_d", [BS * N, D], F32).ap()
    s_d = nc.dram_tensor("s_d", [BS * N, D], F32).ap()

    with tile.TileContext(nc) as tc:
        with (
            tc.tile_pool(name="wpool", bufs=1) as wpool,
            tc.tile_pool(name="stream", bufs=4) as stream,
            tc.tile_pool(name="prod", bufs=2) as prodp,
            tc.tile_pool(name="work", bufs=2) as work,
            tc.tile_pool(name="small", bufs=1) as small,
        ):
          with (
            tc.tile_pool(name="wpre", bufs=1) as wpre,
            tc.tile_pool(name="pwork", bufs=1) as pwork,
            tc.tile_pool(name="psA", bufs=2, space="PSUM") as psA,
            tc.tile_pool(name="psB", bufs=2, space="PSUM") as psB,
            tc.tile_pool(name="psT", bufs=2, space="PSUM") as psT,
          ):
            # ---------------- precompute-only weights ----------------
            wn_sb = wpre.tile([128, 2, 3 * D], F32)
            nc.sync.dma_start(wn_sb[:, 0, :], wnode_in[0:128, :])
            nc.sync.dma_start(wn_sb[:, 1, :], wnode_in[128:256, :])
            ws2_sb = wpre.tile([128, 2, D], F32)       # W_step[256:512]
            nc.sync.dma_start(ws2_sb[:, 0, :], wstep_in[256:384, :])
            nc.sync.dma_start(ws2_sb[:, 1, :], wstep_in[384:512, :])
            ws1_sb = wpre.tile([128, 2, D], F32)       # W_step[0:256]
            nc.sync.dma_start(ws1_sb[:, 0, :], wstep_in[0:128, :])
            nc.sync.dma_start(ws1_sb[:, 1, :], wstep_in[128:256, :])
            wf_sb = wpre.tile([128, 2, D], F32)
            nc.sync.dma_start(wf_sb[:, 0, :], wfix_in[0:128, :])
            nc.sync.dma_start(wf_sb[:, 1, :], wfix_in[128:256, :])
            wo_sb = wpre.tile([128, 2, D], F32)        # W_out row-tiles
            nc.sync.dma_start(wo_sb[:, 0, :], wout_in[0:128, :])
            nc.sync.dma_start(wo_sb[:, 1, :], wout_in[128:256, :])

            pref_sb = wpre.tile([128, D], F32)
            nc.sync.dma_start(
                pref_sb[:],
                pref_in.rearrange("(o f) -> o f", o=1).broadcast_to([128, D]),
            )

            ident = wpre.tile([128, 128], F32)
            io_c = wpre.tile([128, 128], dt.int32)
            nc.gpsimd.iota(io_c[:], pattern=[[1, 128]], channel_multiplier=0)
            io_r = wpre.tile([128, 1], dt.int32)
            nc.gpsimd.iota(io_r[:], pattern=[[0, 1]], channel_multiplier=1)
            id_i = wpre.tile([128, 128], dt.int32)
            nc.vector.tensor_tensor(id_i[:], io_c[:], io_r[:].broadcast_to([128, 128]), op=OP.is_equal)
            nc.vector.tensor_copy(ident[:], id_i[:])

            iota_n = wpool.tile([128, N], dt.int32)
            nc.gpsimd.iota(iota_n[:], pattern=[[1, N]], channel_multiplier=0)
            iota_row = wpool.tile([128, 1], dt.int32)   # p*N
            nc.gpsimd.iota(iota_row[:], pattern=[[0, 1]], channel_multiplier=N)

            amask = wpool.tile([128, N], F32)
            nc.vector.memset(amask[:], 0.0)
            nc.vector.memset(amask[:, START:START + 1], NEG)

            sel = wpool.tile([128, 1], dt.int32)
            selF = wpool.tile([128, 1], F32)
            nc.vector.memset(selF[:], float(START))
            nc.vector.tensor_copy(sel[:], selF[:])

            fixed2 = wpool.tile([128, D], F32)

            # W_out^T in SBUF: woT2[:, jt, i] = W_out[i, j]
            woT2 = wpre.tile([128, 2, D], F32)
            for jt in range(2):
                for it in range(2):
                    tp = psT.tile([128, 128], F32, tag="tp")
                    nc.tensor.transpose(tp[:], wo_sb[:, it, jt * 128:(jt + 1) * 128], ident[:])
                    nc.scalar.copy(woT2[:, jt, it * 128:(it + 1) * 128], tp[:])


            # ---------------- precompute row-tile loop ----------------
            emb_rows = emb_in.rearrange("b n c -> (b n) c")

            def dhv(t2):
                # natural (h,e)-major [p, 256] -> [p, e, h] view (reorder cols)
                return t2.rearrange("p (h e) -> p e h", h=H)

            def ehs(t2):
                # contiguous (e,h)-major [p, 256] -> [p, e, h] view (plain split)
                return t2.rearrange("p (e h) -> p e h", h=H)

            def pre_body(rt):
                r0 = rt * 128
                e2 = pwork.tile([128, D], F32, tag="e2")
                nc.sync.dma_start(e2[:], emb_rows[bass.ds(r0, 128), :])
                nc.vector.tensor_tensor(e2[:], e2[:], pref_sb[:], op=OP.add)
                nc.sync.dma_start(e2_d[bass.ds(r0, 128), :], e2[:])
                e2T = pwork.tile([128, 2, 128], F32, tag="e2T")
                for ci in range(2):
                    tp = psT.tile([128, 128], F32, tag="tp")
                    nc.tensor.transpose(tp[:], e2[:, ci * 128:(ci + 1) * 128], ident[:])
                    nc.scalar.copy(e2T[:, ci, :], tp[:])
                # kvl = e2 @ W_node : psum [512] + [256]
                pa = psA.tile([128, 512], F32, tag="pa")
                nc.tensor.matmul(pa[:], e2T[:, 0, :], wn_sb[:, 0, 0:512], start=True, stop=False)
                nc.tensor.matmul(pa[:], e2T[:, 1, :], wn_sb[:, 1, 0:512], start=False, stop=True)
                pb = psB.tile([128, D], F32, tag="pbx")
                nc.tensor.matmul(pb[:], e2T[:, 0, :], wn_sb[:, 0, 512:768], start=True, stop=False)
                nc.tensor.matmul(pb[:], e2T[:, 1, :], wn_sb[:, 1, 512:768], start=False, stop=True)
                # K row (scaled, (d,h)-major) and V row
                krow = pwork.tile([128, D], F32, tag="krow")
                nc.scalar.activation(krow[:], pa[:, 0:256], ACTF.Copy, scale=ISD)
                nc.sync.dma_start(krow_d[bass.ds(r0, 128), :], krow[:])
                vrow = pwork.tile([128, D], F32, tag="krow")
                nc.vector.tensor_copy(vrow[:], pa[:, 256:512])
                nc.sync.dma_start(vrow_d[bass.ds(r0, 128), :], vrow[:])
                # lK' = (lK @ W_out^T) * ISD256, (d,h)-major via rhs view
                lrow = pwork.tile([128, D], F32, tag="lrow")
                nc.scalar.copy(lrow[:], pb[:])
                lrT = pwork.tile([128, 2, 128], F32, tag="lrT")
                for ci in range(2):
                    tp = psT.tile([128, 128], F32, tag="tp")
                    nc.tensor.transpose(tp[:], lrow[:, ci * 128:(ci + 1) * 128], ident[:])
                    nc.scalar.copy(lrT[:, ci, :], tp[:])
                pc = psB.tile([128, D], F32, tag="pbx")
                nc.tensor.matmul(pc[:], lrT[:, 0, :], woT2[:, 0, :], start=True, stop=False)
                nc.tensor.matmul(pc[:], lrT[:, 1, :], woT2[:, 1, :], start=False, stop=True)
                lprow = pwork.tile([128, D], F32, tag="krow")
                nc.scalar.activation(lprow[:], pc[:], ACTF.Copy, scale=ISD256)
                nc.sync.dma_start(lrow_d[bass.ds(r0, 128), :], lprow[:])
                # S row = e2 @ W_step[256:], (d,h)-major
                pdm = psB.tile([128, D], F32, tag="pbx")
                nc.tensor.matmul(pdm[:], e2T[:, 0, :], ws2_sb[:, 0, :], start=True, stop=False)
                nc.tensor.matmul(pdm[:], e2T[:, 1, :], ws2_sb[:, 1, :], start=False, stop=True)
                srow = pwork.tile([128, D], F32, tag="lrow")
                nc.vector.tensor_copy(srow[:], pdm[:])
                nc.sync.dma_start(s_d[bass.ds(r0, 128), :], srow[:])

            tc.For_i_unrolled(0, ROWT, 1, pre_body, max_unroll=2)

            # -------- fixed2 = mean(e2) @ Wf + e2[:,24] @ Ws1, (d,h)-major --------
            macc = wpool.tile([128, D], F32)
            e2_bnc = e2_d.rearrange("(b n) c -> b n c", b=BS)
            for c in range(N // MC):
                ech = stream.tile([128, MC, D], F32, tag="ch")
                nc.sync.dma_start(ech[:], e2_bnc[:, c * MC:(c + 1) * MC, :])
                part = pwork.tile([128, D], F32, tag="e2")
                nc.vector.tensor_reduce(part[:], ech[:].transpose([0, 2, 1]), axis=AX.X, op=OP.add)
                if c == 0:
                    nc.vector.tensor_copy(macc[:], part[:])
                else:
                    nc.vector.tensor_tensor(macc[:], macc[:], part[:], op=OP.add)
            nc.vector.tensor_scalar(macc[:], macc[:], 1.0 / N, None, op0=OP.mult)
            first_sb = wpool.tile([128, D], F32)
            nc.sync.dma_start(first_sb[:], e2_bnc[:, START, :])

            fT = pwork.tile([128, 2, 128], F32, tag="e2T")
            mT = pwork.tile([128, 2, 128], F32, tag="lrT")
            for ci in range(2):
                tp = psT.tile([128, 128], F32, tag="tp")
                nc.tensor.transpose(tp[:], macc[:, ci * 128:(ci + 1) * 128], ident[:])
                nc.scalar.copy(mT[:, ci, :], tp[:])
                tp2 = psT.tile([128, 128], F32, tag="tp")
                nc.tensor.transpose(tp2[:], first_sb[:, ci * 128:(ci + 1) * 128], ident[:])
                nc.scalar.copy(fT[:, ci, :], tp2[:])
            pf = psA.tile([128, 512], F32, tag="pa")
            nc.tensor.matmul(pf[:, 0:256], mT[:, 0, :], wf_sb[:, 0, :], start=True, stop=False)
            nc.tensor.matmul(pf[:, 0:256], mT[:, 1, :], wf_sb[:, 1, :], start=False, stop=False)
            nc.tensor.matmul(pf[:, 0:256], fT[:, 0, :], ws1_sb[:, 0, :], start=False, stop=False)
            nc.tensor.matmul(pf[:, 0:256], fT[:, 1, :], ws1_sb[:, 1, :], start=False, stop=True)
            nc.vector.tensor_copy(fixed2[:], pf[:, 0:256])

            krow_bn = krow_d.rearrange("(b n) c -> b n c", b=BS)

          # ---------------- decode steps (fp32; DVE/POOL split) ----------------
          if True:
            compat = wpool.tile([128, N, H], F32)
            attn = wpool.tile([128, N, H], F32)
            logits = wpool.tile([128, N], F32)
            gacc = wpool.tile([128, D], F32)
            qf = wpool.tile([128, D], F32)

            vrow_bn = vrow_d.rearrange("(b n) c -> b n c", b=BS)
            lrow_bn = lrow_d.rearrange("(b n) c -> b n c", b=BS)
            NDV = 5          # chunks 0..NDV-1 on DVE, rest on POOL

            def eng(c):
                return nc.vector if c < NDV else nc.gpsimd

            def step_body(t):
                # q = fixed2 + S[prev]
                offs = small.tile([128, 1], dt.int32, tag="offs")
                nc.vector.tensor_tensor(offs[:], iota_row[:], sel[:], op=OP.add)
                srow = small.tile([128, D], F32, tag="sgath")
                nc.gpsimd.indirect_dma_start(
                    out=srow[:], out_offset=None,
                    in_=s_d, in_offset=IndirectOffsetOnAxis(ap=offs[:], axis=0))
                nc.vector.tensor_tensor(qf[:], fixed2[:], srow[:], op=OP.add)
                qbb = qf[:].rearrange("p (n c) -> p n c", n=1).broadcast_to([128, NC, D])

                # ---- compat[n,h] = sum_e K[n,h,e]*q[h,e] ----
                for c in range(NCH):
                    n0 = c * NC
                    kch = stream.tile([128, NC, D], F32, tag="ch")
                    nc.sync.dma_start(kch[:], krow_bn[:, n0:n0 + NC, :])
                    pr = prodp.tile([128, NC, D], F32, tag="pr")
                    eng(c).tensor_tensor(pr[:], kch[:], qbb, op=OP.mult)
                    nc.vector.tensor_reduce(
                        compat[:, n0:n0 + NC, :],
                        pr[:].rearrange("p n (h e) -> p n h e", h=H),
                        axis=AX.X, op=OP.add)
                # softmax over n per h (max-sub for safety)
                nc.vector.tensor_tensor(
                    compat[:], compat[:],
                    amask[:].rearrange("p (n o) -> p n o", o=1).broadcast_to([128, N, H]),
                    op=OP.add)
                cmax = small.tile([128, H], F32, tag="cmax")
                nc.vector.tensor_reduce(cmax[:], compat[:].transpose([0, 2, 1]), axis=AX.X, op=OP.max)
                nc.vector.tensor_tensor(
                    compat[:], compat[:],
                    cmax[:].rearrange("p (o h) -> p o h", o=1).broadcast_to([128, N, H]),
                    op=OP.subtract)
                nc.scalar.activation(attn[:], compat[:], ACTF.Exp)
                ssum = small.tile([128, H], F32, tag="ssum")
                nc.vector.tensor_reduce(ssum[:], attn[:].transpose([0, 2, 1]), axis=AX.X, op=OP.add)
                rh = small.tile([128, H], F32, tag="rh")
                nc.vector.reciprocal(rh[:], ssum[:])
                nc.vector.tensor_tensor(
                    attn[:], attn[:],
                    rh[:].rearrange("p (o h) -> p o h", o=1).broadcast_to([128, N, H]),
                    op=OP.mult)

                # ---- glimpse[h,e] = sum_n V[n,h,e]*attn[n,h] ----
                for c in range(NCH):
                    n0 = c * NC
                    vch = stream.tile([128, NC, D], F32, tag="ch")
                    nc.sync.dma_start(vch[:], vrow_bn[:, n0:n0 + NC, :])
                    pr2 = prodp.tile([128, NC, D], F32, tag="pr")
                    eng(c).tensor_tensor(
                        pr2[:].rearrange("p n (h e) -> p n h e", h=H),
                        vch[:].rearrange("p n (h e) -> p n h e", h=H),
                        attn[:, n0:n0 + NC, :].rearrange("p n (h o) -> p n h o", o=1)
                            .broadcast_to([128, NC, H, dd]),
                        op=OP.mult)
                    # in-place halving tree over n (contiguous reads)
                    nc.vector.tensor_tensor(pr2[:, 0:4], pr2[:, 0:4], pr2[:, 16:20], op=OP.add)
                    w = 16
                    while w > 1:
                        w //= 2
                        nc.vector.tensor_tensor(pr2[:, 0:w], pr2[:, 0:w], pr2[:, w:2 * w], op=OP.add)
                    if c == 0:
                        nc.vector.tensor_copy(gacc[:], pr2[:, 0, :])
                    else:
                        nc.vector.tensor_tensor(gacc[:], gacc[:], pr2[:, 0, :], op=OP.add)
                gbb = gacc[:].rearrange("p (n c) -> p n c", n=1).broadcast_to([128, NC, D])

                # ---- logits[n] = sum_c lK'[n,c]*g[c] ----
                for c in range(NCH):
                    n0 = c * NC
                    lch = stream.tile([128, NC, D], F32, tag="ch")
                    nc.sync.dma_start(lch[:], lrow_bn[:, n0:n0 + NC, :])
                    pr3 = prodp.tile([128, NC, D], F32, tag="pr")
                    eng(c).tensor_tensor(pr3[:], lch[:], gbb, op=OP.mult)
                    nc.vector.tensor_reduce(
                        logits[:, n0:n0 + NC], pr3[:], axis=AX.X, op=OP.add)

                # ---- tanh clip, mask, store (host does -logsumexp) ----
                tnh = work.tile([128, N], F32, tag="tnh")
                nc.scalar.activation(tnh[:], logits[:], ACTF.Tanh)
                lm = work.tile([128, N], F32, tag="lm")
                nc.vector.tensor_scalar(lm[:], tnh[:], 10.0, None, op0=OP.mult)
                nc.vector.tensor_tensor(lm[:], lm[:], amask[:], op=OP.add)
                nc.sync.dma_start(out[:, bass.ds(t * N, N)], lm[:])

                # ---- argmax + state update ----
                mx8 = small.tile([128, 8], F32, tag="mx8")
                nc.vector.max(mx8[:], lm[:])
                ix8 = small.tile([128, 8], dt.uint32, tag="ix8")
                nc.vector.max_index(ix8[:], mx8[:], lm[:])
                nc.vector.tensor_copy(sel[:], ix8[:, 0:1])
                ohi = small.tile([128, N], dt.int32, tag="ohi")
                nc.vector.tensor_tensor(ohi[:], iota_n[:], sel[:].broadcast_to([128, N]), op=OP.is_equal)
                ohf = small.tile([128, N], F32, tag="ohf")
                nc.vector.tensor_copy(ohf[:], ohi[:])
                nc.vector.scalar_tensor_tensor(
                    amask[:], ohf[:], NEG, amask[:], op0=OP.mult, op1=OP.add)

            if DBG:
                step_body(0)
                nc.sync.dma_start(dbg_fixed2[:, :], fixed2[:])
                nc.sync.dma_start(dbg_q[:, :], qf[:])
                nc.sync.dma_start(dbg_compat[:, :], compat[:].rearrange("p n h -> p (n h)"))
                nc.vector.tensor_copy(compat[:], attn[:])
                nc.sync.dma_start(dbg_attn[:, :], compat[:].rearrange("p n h -> p (n h)"))
                nc.vector.tensor_copy(qf[:], gacc[:])
                nc.sync.dma_start(dbg_g[:, :], qf[:])
                nc.sync.dma_start(dbg_logits[:, :], logits[:])
                nc.sync.dma_start(dbg_sel[:, :], sel[:])
                kch0 = stream.tile([128, NC, D], F32, tag="ch")
                nc.sync.dma_start(kch0[:], krow_bn[:, 20:40, :])
                nc.vector.tensor_copy(qf[:], kch0[:, 10, :])
                nc.sync.dma_start(dbg_krow[:, :], qf[:])
                lch0 = stream.tile([128, NC, D], F32, tag="ch")
                nc.sync.dma_start(lch0[:], lrow_bn[:, 20:40, :])
                nc.vector.tensor_copy(qf[:], lch0[:, 10, :])
                nc.sync.dma_start(dbg_lrow[:, :], qf[:])
                sr0 = small.tile([128, D], F32, tag="sgath")
                nc.sync.dma_start(sr0[:], s_d.rearrange("(b n) c -> b n c", b=BS)[:, 24, :])
                nc.sync.dma_start(dbg_srow[:, :], sr0[:])
            else:
                tc.For_i_unrolled(0, T, 1, step_body, max_unroll=2)

    nc.compile()
    return nc


_CACHE = {}


def kernel(**inputs) -> np.ndarray:
    if "nc" not in _CACHE:
        _CACHE["nc"] = _build()
    nc = _CACHE["nc"]

    emb = np.ascontiguousarray(np.asarray(inputs["embeddings"], np.float32))
    shared = {
        "pref_embed": np.asarray(inputs["pref_embed"], np.float32),
        "W_node": np.asarray(inputs["W_node"], np.float32),
        "W_fixed": np.asarray(inputs["W_fixed"], np.float32),
        "W_step": np.asarray(inputs["W_step"], np.float32),
        "W_out": np.asarray(inputs["W_out"], np.float32),
    }
    in_maps = []
    for i in range(NCORES):
        m = {"embeddings": emb[i * BS:(i + 1) * BS]}
        m.update(shared)
        in_maps.append(m)

    res = run_bass_kernel_spmd(nc, in_maps, list(range(NCORES)))
    outs = [res.results[i]["log_p"].reshape(BS, T, N) for i in range(NCORES)]
    lm = np.concatenate(outs, axis=0)  # (B, T, N): 10*tanh + mask, pre-normalization
    # host-side log_softmax normalization (exact, float64)
    x = lm.astype(np.float64)
    xf = np.where(x > -1e8, x, -np.inf)
    mx = xf.max(axis=2, keepdims=True)
    lse = mx + np.log(np.exp(xf - mx).sum(axis=2, keepdims=True))
    return (x - lse).astype(np.float32)


if __name__ == "__main__":
    z = np.load("inputs.npz")
    inp = {k: z[k] for k in z.files}
    o = kernel(**inp)
    print("kernel output", o.shape, o.dtype)
    np.save("kernel_out.npy", o)
